# revision 1
# baseline (speedup 1.0000x reference)
"""Trainium2 Bass kernel for MultiHeadSelfAttention (GroupNorm + QKV + attention + proj + residual).

Problem shape (hardcoded): x [8, 512, 32, 32] fp32, 8 heads, 32 groups.
Sharding: data-parallel over batch B=8 across the 8 NeuronCores (one batch per core).

Per-core pipeline (T = 32*32 = 1024 positions, C = 512 channels, ch = 64 per head):
  1. GroupNorm(32) over [C, T]: per-channel bn_stats, group-combine via tiny PE
     matmuls with a group-indicator matrix, rsqrt via ACT ln/exp, affine fold.
  2. qkv = qkv_w @ h + b computed in two custom layouts (host-reordered weights):
       - q,k: [1024, T] with head-pair packing (k-pair tile, q-pair tile per pair)
       - v:   transposed directly, vT [T, 512], with a ones-rider column per head
  3. Per head: wT[s,t] = k^T q on PE (no transposes needed in this orientation),
     exp on ACT (softmax max-subtraction skipped: logits are O(1) by construction),
     AV matmul with ones-rider to get softmax sums for free, reciprocal +
     gpsimd partition_broadcast + DVE mul to normalize.
  4. proj matmul + bias + residual fused via the AFFINE_THEN_ADD custom DVE op.

All matmuls run in float32r (full PE rate, ~1.5e-4 rms rel err).
"""

import ml_dtypes
import numpy as np

import concourse.bass as bass
import concourse.bacc as bacc
import concourse.tile as tile
import concourse.mybir as mybir
from concourse import library_config
from concourse.bass_utils import run_bass_kernel_spmd
from concourse.dve_ops import AFFINE_THEN_ADD

B, C, HS, WS = 8, 512, 32, 32
T = HS * WS            # 1024
H = 8                  # heads
CH = C // H            # 64
G = 32                 # groups
CPG = C // G           # 16 channels per group
EPS = 1e-5
NCHUNK = C // 128      # 4 channel chunks
NT = T // 128          # 8 sequence tiles
NB = T // 512          # 2 psum banks over T
F32 = mybir.dt.float32
F32R = mybir.dt.float32r
BF16 = mybir.dt.bfloat16

_CACHE = {}
_DEBUG = False


def _orig_row(kind, h, i):
    # row in qkv_w for head h, kind q/k/v, within-head index i
    off = {"q": 0, "k": CH, "v": 2 * CH}[kind]
    return 192 * h + off + i


def _host_weights(gn_w, gn_b, qkv_w, qkv_b, proj_w, proj_b):
    scale2 = 1.0 / np.sqrt(CH)  # ch**-0.25 applied to both q and k -> fold into k
    # qk weights: col layout m-tile 2p = [k_h0 | k_h1], m-tile 2p+1 = [q_h0 | q_h1]
    rows = np.zeros(2 * C, dtype=np.int64)
    colscale = np.ones(2 * C, dtype=np.float32)
    for p in range(H // 2):
        for slot in range(2):
            h = 2 * p + slot
            for i in range(CH):
                col_k = (2 * p) * 128 + slot * CH + i
                rows[col_k] = _orig_row("k", h, i)
                colscale[col_k] = scale2
                col_q = (2 * p + 1) * 128 + slot * CH + i
                rows[col_q] = _orig_row("q", h, i)
    wqk = (qkv_w[rows, :] * colscale[:, None]).T.copy()      # [512, 1024]
    wqk_t = np.ascontiguousarray(
        wqk.reshape(NCHUNK, 128, 2 * C)).astype(np.float32)  # [4, 128, 1024]
    bqk = (qkv_b[rows] * colscale).reshape(8, 128).T.copy()  # [128, 8]

    vrows = np.array([_orig_row("v", h, i) for h in range(H) for i in range(CH)])
    wv = qkv_w[vrows, :].T.copy()                            # [512, 512] (c, c_v)
    wv_t = np.ascontiguousarray(wv.reshape(NCHUNK, 128, C)).astype(np.float32)
    bv = qkv_b[vrows].reshape(1, C).astype(np.float32)

    wproj = proj_w.T.copy()                                  # [512(c), 512(o)]
    wproj_t = np.ascontiguousarray(wproj.reshape(NCHUNK, 128, C)).astype(np.float32)
    bproj = proj_b.reshape(NCHUNK, 128).T.copy()             # [128, 4]

    gnw = gn_w.reshape(NCHUNK, 128).T.copy()                 # [128, 4]
    gnb = gn_b.reshape(NCHUNK, 128).T.copy()

    riderpad = np.zeros((128, H, CH), dtype=np.float32)
    riderpad[:, :, 0] = 1.0
    riderpad = riderpad.astype(ml_dtypes.bfloat16)
    g_all = np.zeros((128, 128), dtype=np.float32)           # [u, 32k+g] = 1/16
    gt_all = np.zeros((32, 512), dtype=np.float32)           # [g, 128k+u] = 1
    for k in range(NCHUNK):
        for u in range(128):
            g = 8 * k + u // CPG
            g_all[u, 32 * k + g] = 1.0 / CPG
            gt_all[g, 128 * k + u] = 1.0
    return {
        "wqk": wqk_t, "bqk": bqk, "wv": wv_t, "bv": bv,
        "wproj": wproj_t, "bproj": bproj, "gnw": gnw, "gnb": gnb,
        "g_all": g_all, "gt_all": gt_all,
        "riderpad": riderpad,
        "ones1": np.ones((1, 128), dtype=np.float32),
    }


def _build_program(n_reps=1, pa_bufs=2, pb_bufs=4, ew_bufs=18, qk_bufs=4, big_exp=False, pw_bufs=2, qk_act=(0, 1)):
    nc = bacc.Bacc("TRN2", target_bir_lowering=False, debug=False, num_devices=8)
    dt_in = [
        ("x", [C, T], F32), ("wqk", [NCHUNK, 128, 2 * C], F32R),
        ("bqk", [128, 8], F32), ("wv", [NCHUNK, 128, C], F32R),
        ("bv", [1, C], F32R), ("wproj", [NCHUNK, 128, C], F32R),
        ("bproj", [128, NCHUNK], F32), ("gnw", [128, NCHUNK], F32),
        ("gnb", [128, NCHUNK], F32), ("g_all", [128, 128], F32R),
        ("gt_all", [32, 512], F32R), ("riderpad", [128, H, CH], BF16),
        ("ones1", [1, 128], F32R),
    ]
    d = {name: nc.dram_tensor(name, shape, dt, kind="ExternalInput").ap()
         for name, shape, dt in dt_in}
    out_d = nc.dram_tensor("out", [C, T], F32, kind="ExternalOutput").ap()
    if _DEBUG:
        a_o = nc.dram_tensor("a_o", [C, T], F32R, kind="ExternalOutput").ap()
        rec_o = nc.dram_tensor("rec_o", [H, T], F32, kind="ExternalOutput").ap()
        rb_o = nc.dram_tensor("rb_o", [H, T], F32, kind="ExternalOutput").ap()

    with tile.TileContext(nc) as tc:
        with (
            tc.tile_pool(name="singles", bufs=1) as singles,
            tc.tile_pool(name="small", bufs=10) as small,
            tc.tile_pool(name="qkp", bufs=qk_bufs) as qkp,
            tc.tile_pool(name="ewp", bufs=ew_bufs) as ewp,
            tc.tile_pool(name="recp", bufs=3) as recp,
            tc.tile_pool(name="rbp", bufs=3) as rbp,
            tc.tile_pool(name="outp", bufs=2) as outp,
            tc.tile_pool(name="pA", bufs=pa_bufs, space="PSUM") as pA,
            tc.tile_pool(name="pB", bufs=(pw_bufs if big_exp else pb_bufs),
                         space="PSUM") as pB,
        ):
            nc.gpsimd.load_library(library_config.attn)

            # ---- load inputs: x + GN-critical consts first (sync queue),
            # ---- bulk weights for later phases on the gpsimd queue ----
            x_sb = []
            for k in range(NCHUNK):
                t_ = singles.tile([128, T], F32, tag=f"x{k}")
                nc.sync.dma_start(t_[:], d["x"][128 * k:128 * (k + 1), :])
                x_sb.append(t_)
            g_sb = singles.tile([128, 128], F32R, tag="g_all")
            nc.sync.dma_start(g_sb[:], d["g_all"][:])
            gt_sb = singles.tile([32, 512], F32R, tag="gt_all")
            nc.sync.dma_start(gt_sb[:], d["gt_all"][:])
            gnw_sb = singles.tile([128, NCHUNK], F32, tag="gnw")
            nc.sync.dma_start(gnw_sb[:], d["gnw"][:])
            gnb_sb = singles.tile([128, NCHUNK], F32, tag="gnb")
            nc.sync.dma_start(gnb_sb[:], d["gnb"][:])
            bqk_sb = singles.tile([128, 8], F32, tag="bqk")
            nc.sync.dma_start(bqk_sb[:], d["bqk"][:])
            wqk_sb = []
            for k in range(NCHUNK):
                t_ = singles.tile([128, 2 * C], F32R, tag=f"wqk{k}")
                nc.sync.dma_start(t_[:], d["wqk"][k])
                wqk_sb.append(t_)
            eps_t = singles.tile([32, 1], F32, tag="eps")
            nc.vector.memset(eps_t[:], EPS)
            wv_sb = []
            for k in range(NCHUNK):
                t_ = singles.tile([128, C], F32R, tag=f"wv{k}")
                nc.gpsimd.dma_start(t_[:], d["wv"][k])
                wv_sb.append(t_)
            wproj_sb = []
            for k in range(NCHUNK):
                t_ = singles.tile([128, C], F32R, tag=f"wproj{k}")
                nc.gpsimd.dma_start(t_[:], d["wproj"][k])
                wproj_sb.append(t_)
            ones1_sb = singles.tile([1, 128], F32R, tag="ones1")
            nc.gpsimd.dma_start(ones1_sb[:], d["ones1"][:])
            bv_sb = singles.tile([1, C], F32R, tag="bv")
            nc.gpsimd.dma_start(bv_sb[:], d["bv"][:])
            bproj_sb = singles.tile([128, NCHUNK], F32, tag="bproj")
            nc.gpsimd.dma_start(bproj_sb[:], d["bproj"][:])

            for rep in range(n_reps):
                sfx = f"r{rep}"
                # ================= GroupNorm =================
                h_sb = []
                psum_gs = pB.tile([32, 2], F32, tag="pB", name="psum_gs")
                stats_list = []
                for k in range(NCHUNK):
                    st6 = small.tile([128, 2, 6], F32, tag="small")
                    nc.vector.bn_stats(st6[:, 0, :], x_sb[k][:, 0:512])
                    nc.vector.bn_stats(st6[:, 1, :], x_sb[k][:, 512:1024])
                    mv = small.tile([128, 2], F32, tag="small")
                    nc.vector.bn_aggr(mv[:], st6[:])
                    m2 = small.tile([128, 1], F32, tag="small")
                    nc.vector.tensor_mul(m2[:], mv[:, 0:1], mv[:, 0:1])
                    stats = small.tile([128, 2], F32R, tag="small")
                    nc.vector.tensor_copy(stats[:, 0:1], mv[:, 0:1])
                    nc.vector.tensor_add(stats[:, 1:2], mv[:, 1:2], m2[:])
                    stats_list.append(stats)
                for k in range(NCHUNK):
                    nc.tensor.matmul(psum_gs[:], g_sb[:, 32 * k:32 * (k + 1)],
                                     stats_list[k][:], start=(k == 0), stop=(k == 3))
                gsb = small.tile([32, 2], F32, tag="small")
                nc.vector.tensor_copy(gsb[:], psum_gs[:])
                mu2 = small.tile([32, 1], F32, tag="small")
                nc.vector.tensor_mul(mu2[:], gsb[:, 0:1], gsb[:, 0:1])
                varg = small.tile([32, 1], F32, tag="small")
                nc.vector.tensor_sub(varg[:], gsb[:, 1:2], mu2[:])
                lnv = small.tile([32, 1], F32, tag="small")
                nc.scalar.activation(lnv[:], varg[:], mybir.ActivationFunctionType.Ln,
                                     bias=eps_t[:], scale=1.0)
                rstd = small.tile([32, 1], F32, tag="small")
                nc.scalar.activation(rstd[:], lnv[:], mybir.ActivationFunctionType.Exp,
                                     scale=-0.5)
                grp = small.tile([32, 2], F32R, tag="small")
                nc.vector.tensor_copy(grp[:, 0:1], gsb[:, 0:1])
                nc.vector.tensor_copy(grp[:, 1:2], rstd[:])
                for k in range(NCHUNK):
                    psum_pc = pB.tile([128, 2], F32, tag="pB", name="psum_pc")
                    nc.tensor.matmul(psum_pc[:], gt_sb[:, 128 * k:128 * (k + 1)],
                                     grp[:], start=True, stop=True)
                    s_c = small.tile([128, 1], F32, tag="small")
                    nc.vector.tensor_mul(s_c[:], psum_pc[:, 1:2], gnw_sb[:, k:k + 1])
                    t1 = small.tile([128, 1], F32, tag="small")
                    nc.vector.tensor_mul(t1[:], psum_pc[:, 0:1], s_c[:])
                    b_c = small.tile([128, 1], F32, tag="small")
                    nc.vector.tensor_sub(b_c[:], gnb_sb[:, k:k + 1], t1[:])
                    ht = singles.tile([128, T], F32R, tag=f"h{k}", name=f"h{k}")
                    nc.vector.tensor_scalar(
                        out=ht[:], in0=x_sb[k][:], scalar1=s_c[:], scalar2=b_c[:],
                        op0=mybir.AluOpType.mult, op1=mybir.AluOpType.add)
                    h_sb.append(ht)

                # ============ vT = (h^T @ Wv)^T with bias rider (lazy) ======
                def emit_vt_tile(mt):
                    pv = pB.tile([128, C], F32, tag="pB", name="pv")
                    for k in range(NCHUNK):
                        nc.tensor.matmul(pv[:],
                                         h_sb[k][:, 128 * mt:128 * (mt + 1)],
                                         wv_sb[k][:], start=(k == 0), stop=False)
                    nc.tensor.matmul(pv[:], ones1_sb[:], bv_sb[:],
                                     start=False, stop=True)
                    vt = singles.tile([128, H, 2 * CH], BF16, tag=f"vt{mt}",
                                      name=f"vt{mt}")
                    nc.sync.dma_start(vt[:, :, 0:CH], d["riderpad"][:])
                    nc.vector.tensor_copy(
                        vt[:, :, CH:2 * CH],
                        pv[:].rearrange("p (h c) -> p h c", h=H))
                    return vt

                # ================= q,k tiles =================
                def make_qk(m):
                    pq = pA.tile([128, T], F32, tag="pA")
                    for nb in range(NB):
                        for k in range(NCHUNK):
                            nc.tensor.matmul(
                                pq[:, 512 * nb:512 * (nb + 1)],
                                wqk_sb[k][:, 128 * m:128 * (m + 1)],
                                h_sb[k][:, 512 * nb:512 * (nb + 1)],
                                start=(k == 0), stop=(k == 3))
                    qk = qkp.tile([128, T], F32R, tag="qk")
                    if m in qk_act:
                        nc.scalar.activation(
                            qk[:], pq[:], mybir.ActivationFunctionType.Identity,
                            bias=bqk_sb[:, m:m + 1])
                    else:
                        nc.vector.tensor_scalar(
                            out=qk[:], in0=pq[:], scalar1=bqk_sb[:, m:m + 1],
                            scalar2=None, op0=mybir.AluOpType.add)
                    return qk

                # ================= attention =================
                a_sb = [singles.tile([128, T], F32R, tag=f"a{p}",
                                     name=f"a{p}{sfx}")
                        for p in range(NCHUNK)]
                vt_sb = None
                qk_tiles = {}

                def emit_qk_step(h, st, ew_h):
                    # 2 matmuls + 2 exps for head h, s-tile st (both t-banks)
                    p, slot = h // 2, h % 2
                    lo, hi = CH * slot, CH * (slot + 1)
                    ktile, qtile = qk_tiles[2 * p], qk_tiles[2 * p + 1]
                    for nb in range(NB):
                        pw = pB.tile([128, 512], F32, tag="pB", name="pw")
                        nc.tensor.matmul(
                            pw[:], ktile[lo:hi, 128 * st:128 * (st + 1)],
                            qtile[lo:hi, 512 * nb:512 * (nb + 1)],
                            start=True, stop=True)
                        nc.scalar.activation(
                            ew_h[st][:, 512 * nb:512 * (nb + 1)], pw[:],
                            mybir.ActivationFunctionType.Exp)

                def make_qk(m):
                    pq = pA.tile([128, T], F32, tag="pA", name="pq")
                    for nb in range(NB):
                        for k in range(NCHUNK):
                            nc.tensor.matmul(
                                pq[:, 512 * nb:512 * (nb + 1)],
                                wqk_sb[k][:, 128 * m:128 * (m + 1)],
                                h_sb[k][:, 512 * nb:512 * (nb + 1)],
                                start=(k == 0), stop=(k == 3))
                    qk = qkp.tile([128, T], F32R, tag="qk", name="qk")
                    if m in qk_act:
                        nc.scalar.activation(
                            qk[:], pq[:], mybir.ActivationFunctionType.Identity,
                            bias=bqk_sb[:, m:m + 1])
                    else:
                        nc.vector.tensor_scalar(
                            out=qk[:], in0=pq[:], scalar1=bqk_sb[:, m:m + 1],
                            scalar2=None, op0=mybir.AluOpType.add)
                    return qk

                def qk_spread_duty(m):
                    # generator of per-step emissions producing qk tile m
                    # (8 matmuls over 4 steps, then the bias-add)
                    pq = pA.tile([128, T], F32, tag="pA", name="pq")
                    qk = qkp.tile([128, T], F32R, tag="qk", name="qk")
                    qk_tiles[m] = qk
                    for nb in range(NB):
                        for k in range(0, NCHUNK, 2):
                            nc.tensor.matmul(
                                pq[:, 512 * nb:512 * (nb + 1)],
                                wqk_sb[k][:, 128 * m:128 * (m + 1)],
                                h_sb[k][:, 512 * nb:512 * (nb + 1)],
                                start=(k == 0), stop=False)
                            nc.tensor.matmul(
                                pq[:, 512 * nb:512 * (nb + 1)],
                                wqk_sb[k + 1][:, 128 * m:128 * (m + 1)],
                                h_sb[k + 1][:, 512 * nb:512 * (nb + 1)],
                                start=False, stop=(k + 1 == NCHUNK - 1))
                            yield
                    nc.vector.tensor_scalar(
                        out=qk[:], in0=pq[:], scalar1=bqk_sb[:, m:m + 1],
                        scalar2=None, op0=mybir.AluOpType.add)
                    yield

                def proj_partial_duty(p):
                    # pair p's proj contribution: 8 (m, nb) matmul+add units.
                    # p == 0 fuses the bias + residual init via the native
                    # scalar_tensor_tensor ((po + b) + x) — custom DVE ops
                    # mishandle AP offsets so they are avoided on slices.
                    for m in range(NCHUNK):
                        for nb in range(NB):
                            sl = slice(512 * nb, 512 * (nb + 1))
                            po = pB.tile([128, 512], F32, tag="pB", name="po")
                            nc.tensor.matmul(
                                po[:],
                                wproj_sb[p][:, 128 * m:128 * (m + 1)],
                                a_sb[p][:, sl],
                                start=True, stop=True)
                            if p == 0:
                                nc.vector.scalar_tensor_tensor(
                                    out=acc_sb[m][:, sl], in0=po[:],
                                    scalar=bproj_sb[:, m:m + 1],
                                    in1=x_sb[m][:, sl],
                                    op0=mybir.AluOpType.add,
                                    op1=mybir.AluOpType.add)
                            else:
                                nc.vector.tensor_add(
                                    acc_sb[m][:, sl], po[:],
                                    acc_sb[m][:, sl])
                            yield

                ew = {hh: [ewp.tile([128, T], BF16, tag="ew", name=f"ew{hh}")
                           for _ in range(NT)] for hh in range(H)}
                acc_sb = [singles.tile([128, T], F32, tag=f"acc{m}",
                                       name=f"acc{m}{sfx}")
                          for m in range(NCHUNK)]
                qk_tiles[0] = make_qk(0)
                # q tile of pair 0 produced bank-by-bank: QK(0) over t-bank 0
                # starts while q's bank 1 is still in the matmul queue (the
                # k-tile windows span all of t, so k must be complete first)
                pq1 = pA.tile([128, T], F32, tag="pA", name="pq1")
                qk1 = qkp.tile([128, T], F32R, tag="qk", name="qk1")
                qk_tiles[1] = qk1
                for nb in range(NB):
                    sl = slice(512 * nb, 512 * (nb + 1))
                    for k in range(NCHUNK):
                        nc.tensor.matmul(
                            pq1[:, sl], wqk_sb[k][:, 128:256],
                            h_sb[k][:, sl], start=(k == 0), stop=(k == 3))
                    nc.vector.tensor_scalar(
                        out=qk1[:, sl], in0=pq1[:, sl], scalar1=bqk_sb[:, 1:2],
                        scalar2=None, op0=mybir.AluOpType.add)
                    ktile = qk_tiles[0]
                    for st in range(NT):
                        pw = pB.tile([128, 512], F32, tag="pB", name="pw")
                        nc.tensor.matmul(
                            pw[:], ktile[0:CH, 128 * st:128 * (st + 1)],
                            qk1[0:CH, sl], start=True, stop=True)
                        nc.scalar.activation(
                            ew[0][st][:, sl], pw[:],
                            mybir.ActivationFunctionType.Exp)
                vt_sb = [emit_vt_tile(0)]

                for h in range(H):
                    p, slot = h // 2, h % 2
                    lo, hi = CH * slot, CH * (slot + 1)
                    duties = []
                    if slot == 0 and h + 2 < H:
                        # produce next pair's q,k tiles during this head; they
                        # are consumed by QK steps starting at head h+1
                        duties.append(qk_spread_duty(h + 2))
                        duties.append(qk_spread_duty(h + 3))
                    if slot == 1 and p >= 1:
                        duties.append(proj_partial_duty(p - 1))
                    pa = pA.tile([128, T], F32, tag="pA", name="pa")
                    for st in range(NT):
                        if h + 1 < H:
                            emit_qk_step(h + 1, st, ew[h + 1])
                        if h == 0 and st + 1 < NT:
                            vt_sb.append(emit_vt_tile(st + 1))
                        advanced = 0
                        while duties and advanced < 2:
                            try:
                                next(duties[0])
                                advanced += 1
                            except StopIteration:
                                duties.pop(0)
                        for nb in range(NB):
                            nc.tensor.matmul(
                                pa[:, 512 * nb:512 * (nb + 1)],
                                vt_sb[st][:, h, :],
                                ew[h][st][:, 512 * nb:512 * (nb + 1)],
                                start=(st == 0), stop=(st == NT - 1))
                    for g in duties:
                        for _ in g:
                            pass
                    if h == H - 1:
                        for nb in range(NB):
                            sl = slice(512 * nb, 512 * (nb + 1))
                            rcb = recp.tile([1, 512], F32, tag="rcb",
                                            name="rcb")
                            nc.vector.reciprocal_approx_fast(
                                rcb[:], pa[0:1, sl])
                            rbb = rbp.tile([CH, 512], F32, tag="rbb",
                                           name="rbb")
                            nc.gpsimd.partition_broadcast(rbb[:], rcb[:])
                            nc.vector.tensor_mul(
                                a_sb[p][lo:hi, sl], pa[CH:2 * CH, sl], rbb[:])
                    else:
                        rec = recp.tile([1, T], F32, tag="rec")
                        nc.vector.reciprocal_approx_fast(rec[:], pa[0:1, :])
                        rb = rbp.tile([CH, T], F32, tag="rb")
                        nc.gpsimd.partition_broadcast(rb[:], rec[:])
                        nc.vector.tensor_mul(a_sb[p][lo:hi, :],
                                             pa[CH:2 * CH, :], rb[:])
                    rec = None
                    if _DEBUG and rep == n_reps - 1:
                        nc.sync.dma_start(rec_o[h:h + 1, :], rec[:])
                        nc.sync.dma_start(rb_o[h:h + 1, :], rb[0:1, :])

                if _DEBUG and rep == n_reps - 1:
                    for pp in range(NCHUNK):
                        nc.sync.dma_start(a_o[128 * pp:128 * (pp + 1), :],
                                          a_sb[pp][:])

                # ====== tail: pair-3 proj contribution + out ======
                for m in range(NCHUNK):
                    for nb in range(NB):
                        po = pB.tile([128, 512], F32, tag="pB", name="po")
                        nc.tensor.matmul(
                            po[:], wproj_sb[3][:, 128 * m:128 * (m + 1)],
                            a_sb[3][:, 512 * nb:512 * (nb + 1)],
                            start=True, stop=True)
                        ot_slice = acc_sb[m][:, 512 * nb:512 * (nb + 1)]
                        nc.vector.tensor_add(ot_slice, po[:], ot_slice)
                    if rep == n_reps - 1:
                        nc.sync.dma_start(out_d[128 * m:128 * (m + 1), :],
                                          acc_sb[m][:])

    nc.compile()
    return nc


def _get_program(n_reps=1):
    key = ("prog", n_reps)
    if key not in _CACHE:
        _CACHE[key] = _build_program(n_reps)
    return _CACHE[key]


def kernel(x, gn_w, gn_b, qkv_w, qkv_b, proj_w, proj_b, _n_reps=1):
    x = np.asarray(x, dtype=np.float32)
    hw = _host_weights(np.asarray(gn_w, np.float32), np.asarray(gn_b, np.float32),
                       np.asarray(qkv_w, np.float32), np.asarray(qkv_b, np.float32),
                       np.asarray(proj_w, np.float32), np.asarray(proj_b, np.float32))
    xr = np.ascontiguousarray(x.reshape(B, C, T))
    nc = _get_program(_n_reps)
    in_maps = [dict(hw, x=xr[b]) for b in range(B)]
    res = run_bass_kernel_spmd(nc, in_maps, core_ids=list(range(B)))
    out = np.stack([res.results[b]["out"] for b in range(B)])
    return out.reshape(B, C, HS, WS).astype(np.float32)



# revision 5
# speedup vs baseline: 1.0523x; 1.0523x over previous
"""Trainium2 Bass kernel for MultiHeadSelfAttention (GroupNorm + QKV + attention + proj + residual).

Problem shape (hardcoded): x [8, 512, 32, 32] fp32, 8 heads, 32 groups.
Sharding: data-parallel over batch B=8 across the 8 NeuronCores (one batch per core).

Per-core pipeline (T = 1024 positions, C = 512 channels, ch = 64 per head):
  1. GroupNorm(32) chunk-pipelined: groups never cross a 128-channel chunk, so
     each chunk's stats -> rstd -> affine runs as soon as its x tiles land.
  2. qkv = qkv_w @ h with host-reordered bf16 weights:
       - q,k tiles [128, T]: m-tile 2p = [k_h(2p)|k_h(2p+1)], 2p+1 = [q...]
       - v produced transposed per s-tile, packed as fp8e4 pairs for DoubleRow
  3. Per head: logits via PE (f32r/bf16), ONE merged exp [128,1024] per s-tile
     on ACT writing fp8e4 directly; attention @ V via fp8 DoubleRow matmuls
     (2 s-planes per instruction, 16-col rider block carries the softmax
     denominator in partition 0); DVE reciprocal + gpsimd partition_broadcast
     + DVE mul to normalize.
  4. proj accumulated in PSUM per pair-group; v-bias folded into the proj bias
     on the host; bias+residual fused via scalar_tensor_tensor.
"""

import ml_dtypes
import numpy as np

import concourse.bass as bass
import concourse.bacc as bacc
import concourse.tile as tile
import concourse.mybir as mybir
from concourse import library_config
from concourse.bass_utils import run_bass_kernel_spmd

B, C, HS, WS = 8, 512, 32, 32
T = HS * WS            # 1024
H = 8                  # heads
CH = C // H            # 64
G = 32                 # groups
CPG = C // G           # 16 channels per group
EPS = 1e-5
NCHUNK = C // 128      # 4 channel chunks
NT = T // 128          # 8 sequence tiles
NB = T // 512          # 2 psum banks over T
NJ = NT // 2           # 4 s-tile pairs (DoubleRow planes)
RID = 64               # rider cols per head (col 0 = ones); out partitions 64+64=128
                       # (PSUM partition reads must be 0/64-aligned; DR needs 16|M)
F32 = mybir.dt.float32
F32R = mybir.dt.float32r
BF16 = mybir.dt.bfloat16
FP8 = mybir.dt.float8e4
EXP = mybir.ActivationFunctionType.Exp
LN = mybir.ActivationFunctionType.Ln
IDENT = mybir.ActivationFunctionType.Identity
DR = mybir.MatmulPerfMode.DoubleRow

_CACHE = {}


def _orig_row(kind, h, i):
    off = {"q": 0, "k": CH, "v": 2 * CH}[kind]
    return 192 * h + off + i


def _host_weights(gn_w, gn_b, qkv_w, qkv_b, proj_w, proj_b):
    scale2 = 1.0 / np.sqrt(CH)  # ch**-0.25 on both q and k -> fold into k
    rows = np.zeros(2 * C, dtype=np.int64)
    colscale = np.ones(2 * C, dtype=np.float32)
    for p in range(H // 2):
        for slot in range(2):
            h = 2 * p + slot
            for i in range(CH):
                col_k = (2 * p) * 128 + slot * CH + i
                rows[col_k] = _orig_row("k", h, i)
                colscale[col_k] = scale2
                col_q = (2 * p + 1) * 128 + slot * CH + i
                rows[col_q] = _orig_row("q", h, i)
    wqk = (qkv_w[rows, :] * colscale[:, None]).T.copy()      # [512, 1024]
    wqk_t = np.ascontiguousarray(
        wqk.reshape(NCHUNK, 128, 2 * C)).astype(ml_dtypes.bfloat16)
    bqk = (qkv_b[rows] * colscale).reshape(8, 128).T.copy()  # [128, 8]

    vrows = np.array([_orig_row("v", h, i) for h in range(H) for i in range(CH)])
    wv = qkv_w[vrows, :].T.copy()                            # [512, 512] (c, c_v)
    wv_t = np.ascontiguousarray(wv.reshape(NCHUNK, 128, C)).astype(ml_dtypes.bfloat16)

    # v bias folded into proj bias: out = Wp @ (a_hat + bv) + bp
    bv = qkv_b[vrows]
    bproj_full = proj_b + proj_w @ bv                        # [512]
    wproj = proj_w.T.copy()                                  # [512(c), 512(o)]
    wproj_t = np.ascontiguousarray(
        wproj.reshape(NCHUNK, 128, C)).astype(ml_dtypes.bfloat16)
    bproj = bproj_full.reshape(NCHUNK, 128).T.copy().astype(np.float32)  # [128, 4]

    gnw = gn_w.reshape(NCHUNK, 128).T.copy()                 # [128, 4]
    gnb = gn_b.reshape(NCHUNK, 128).T.copy()

    # per-chunk group combine (identical for every chunk: groups = u//16)
    g8 = np.zeros((128, 8), dtype=np.float32)
    gt8 = np.zeros((8, 128), dtype=np.float32)
    for u in range(128):
        g8[u, u // CPG] = 1.0 / CPG
        gt8[u // CPG, u] = 1.0
    return {
        "wqk": wqk_t, "bqk": bqk.astype(np.float32), "wv": wv_t,
        "wproj": wproj_t, "bproj": bproj, "gnw": gnw.astype(np.float32),
        "gnb": gnb.astype(np.float32), "g8": g8, "gt8": gt8,
    }


def _build_program(n_reps=1, ew_bufs=9):
    nc = bacc.Bacc("TRN2", target_bir_lowering=False, debug=False, num_devices=8)
    dt_in = [
        ("x", [C, T], F32), ("wqk", [NCHUNK, 128, 2 * C], BF16),
        ("bqk", [128, 8], F32), ("wv", [NCHUNK, 128, C], BF16),
        ("wproj", [NCHUNK, 128, C], BF16), ("bproj", [128, NCHUNK], F32),
        ("gnw", [128, NCHUNK], F32), ("gnb", [128, NCHUNK], F32),
        ("g8", [128, 8], F32R), ("gt8", [8, 128], F32R),
    ]
    d = {name: nc.dram_tensor(name, shape, dt, kind="ExternalInput").ap()
         for name, shape, dt in dt_in}
    out_d = nc.dram_tensor("out", [C, T], F32, kind="ExternalOutput").ap()

    with tile.TileContext(nc) as tc:
        with (
            tc.tile_pool(name="singles", bufs=1) as singles,
            tc.tile_pool(name="small", bufs=12) as small,
            tc.tile_pool(name="ewp", bufs=ew_bufs) as ewp,
            tc.tile_pool(name="recp", bufs=2) as recp,
            tc.tile_pool(name="rbp", bufs=2) as rbp,
            tc.tile_pool(name="psA", bufs=2, space="PSUM") as psA,
            tc.tile_pool(name="psB", bufs=2, space="PSUM") as psB,
        ):
            nc.gpsimd.load_library(library_config.attn)

            # ---- input DMAs: x halves + wqk interleaved (both needed first);
            # ---- bulk weights on the gpsimd SWDGE queue ----
            x_sb = []
            for k in range(NCHUNK):
                t_ = singles.tile([128, T], F32, tag=f"x{k}", name=f"x{k}")
                x_sb.append(t_)
            wqk_sb = []
            for k in range(NCHUNK):
                t_ = singles.tile([128, 2 * C], BF16, tag=f"wqk{k}",
                                  name=f"wqk{k}")
                wqk_sb.append(t_)
            for k in range(NCHUNK):
                nc.sync.dma_start(x_sb[k][:, 0:512], d["x"][128 * k:128 * (k + 1), 0:512])
                nc.sync.dma_start(x_sb[k][:, 512:1024], d["x"][128 * k:128 * (k + 1), 512:1024])
                nc.scalar.dma_start(wqk_sb[k][:], d["wqk"][k])
            g8_sb = singles.tile([128, 8], F32R, tag="g8", name="g8")
            nc.gpsimd.dma_start(g8_sb[:], d["g8"][:])
            gt8_sb = singles.tile([8, 128], F32R, tag="gt8", name="gt8")
            nc.gpsimd.dma_start(gt8_sb[:], d["gt8"][:])
            gnw_sb = singles.tile([128, NCHUNK], F32, tag="gnw", name="gnw")
            nc.gpsimd.dma_start(gnw_sb[:], d["gnw"][:])
            gnb_sb = singles.tile([128, NCHUNK], F32, tag="gnb", name="gnb")
            nc.gpsimd.dma_start(gnb_sb[:], d["gnb"][:])
            bqk_sb = singles.tile([128, 8], F32, tag="bqk", name="bqk")
            nc.gpsimd.dma_start(bqk_sb[:], d["bqk"][:])
            wv_sb = []
            for k in range(NCHUNK):
                t_ = singles.tile([128, C], BF16, tag=f"wv{k}", name=f"wv{k}")
                nc.gpsimd.dma_start(t_[:], d["wv"][k])
                wv_sb.append(t_)
            wproj_sb = []
            for k in range(NCHUNK):
                t_ = singles.tile([128, C], BF16, tag=f"wproj{k}", name=f"wproj{k}")
                nc.gpsimd.dma_start(t_[:], d["wproj"][k])
                wproj_sb.append(t_)
            bproj_sb = singles.tile([128, NCHUNK], F32, tag="bproj", name="bproj")
            nc.gpsimd.dma_start(bproj_sb[:], d["bproj"][:])
            eps_t = singles.tile([8, 1], F32, tag="eps", name="eps")
            nc.vector.memset(eps_t[:], EPS)

            for rep in range(n_reps):
                sfx = f"r{rep}"
                # ================= GroupNorm (per chunk) =================
                h_sb = []
                for k in range(NCHUNK):
                    st6 = small.tile([128, 2, 6], F32, tag="small", name="st6")
                    nc.vector.bn_stats(st6[:, 0, :], x_sb[k][:, 0:512])
                    nc.vector.bn_stats(st6[:, 1, :], x_sb[k][:, 512:1024])
                    mv = small.tile([128, 2], F32, tag="small", name="mv")
                    nc.vector.bn_aggr(mv[:], st6[:])
                    m2 = small.tile([128, 1], F32, tag="small", name="m2")
                    nc.vector.tensor_mul(m2[:], mv[:, 0:1], mv[:, 0:1])
                    stats = small.tile([128, 2], F32R, tag="small", name="stats")
                    nc.vector.tensor_copy(stats[:, 0:1], mv[:, 0:1])
                    nc.vector.tensor_add(stats[:, 1:2], mv[:, 1:2], m2[:])
                    psg = psB.tile([8, 2], F32, tag="pw", name="psg")
                    nc.tensor.matmul(psg[:], g8_sb[:], stats[:],
                                     start=True, stop=True)
                    gsb = small.tile([8, 2], F32, tag="small", name="gsb")
                    nc.vector.tensor_copy(gsb[:], psg[:])
                    mu2 = small.tile([8, 1], F32, tag="small", name="mu2")
                    nc.vector.tensor_mul(mu2[:], gsb[:, 0:1], gsb[:, 0:1])
                    varg = small.tile([8, 1], F32, tag="small", name="varg")
                    nc.vector.tensor_sub(varg[:], gsb[:, 1:2], mu2[:])
                    lnv = small.tile([8, 1], F32, tag="small", name="lnv")
                    nc.scalar.activation(lnv[:], varg[:], LN, bias=eps_t[:],
                                         scale=1.0)
                    rstd = small.tile([8, 1], F32, tag="small", name="rstd")
                    nc.scalar.activation(rstd[:], lnv[:],
                                         mybir.ActivationFunctionType.Exp,
                                         scale=-0.5)
                    grp = small.tile([8, 2], F32R, tag="small", name="grp")
                    nc.vector.tensor_copy(grp[:, 0:1], gsb[:, 0:1])
                    nc.vector.tensor_copy(grp[:, 1:2], rstd[:])
                    psc = psB.tile([128, 2], F32, tag="pw", name="psc")
                    nc.tensor.matmul(psc[:], gt8_sb[:], grp[:],
                                     start=True, stop=True)
                    s_c = small.tile([128, 1], F32, tag="small", name="s_c")
                    nc.vector.tensor_mul(s_c[:], psc[:, 1:2], gnw_sb[:, k:k + 1])
                    t1 = small.tile([128, 1], F32, tag="small", name="t1")
                    nc.vector.tensor_mul(t1[:], psc[:, 0:1], s_c[:])
                    b_c = small.tile([128, 1], F32, tag="small", name="b_c")
                    nc.vector.tensor_sub(b_c[:], gnb_sb[:, k:k + 1], t1[:])
                    ht = singles.tile([128, T], BF16, tag=f"h{k}", name=f"h{k}")
                    for nb in range(NB):
                        sl = slice(512 * nb, 512 * (nb + 1))
                        nc.vector.tensor_scalar(
                            out=ht[:, sl], in0=x_sb[k][:, sl], scalar1=s_c[:],
                            scalar2=b_c[:], op0=mybir.AluOpType.mult,
                            op1=mybir.AluOpType.add)
                    h_sb.append(ht)

                # ================= qk tiles =================
                qk_tiles = {}

                def gen_qk(m, copy_engines=("v", "v")):
                    pq = psA.tile([128, T], F32, tag="big", name="pq")
                    qk = singles.tile([128, T], BF16, tag=f"qk{m}{sfx}",
                                      name=f"qk{m}")
                    qk_tiles[m] = qk
                    for nb in range(NB):
                        sl = slice(512 * nb, 512 * (nb + 1))
                        for k in range(NCHUNK):
                            nc.tensor.matmul(
                                pq[:, sl], wqk_sb[k][:, 128 * m:128 * (m + 1)],
                                h_sb[k][:, sl], start=(k == 0), stop=(k == 3))
                        if copy_engines[nb] == "a":
                            nc.scalar.activation(qk[:, sl], pq[:, sl], IDENT,
                                                 bias=bqk_sb[:, m:m + 1])
                        else:
                            nc.vector.tensor_scalar(
                                out=qk[:, sl], in0=pq[:, sl],
                                scalar1=bqk_sb[:, m:m + 1], scalar2=None,
                                op0=mybir.AluOpType.add)

                gen_qk(0, ("a", "v"))
                gen_qk(1, ("a", "v"))

                # ================= attention state =================
                ew_pairs = {}

                def emit_qk_step(h, st):
                    # logits for head h, s-tile st: 2 matmuls + 1 merged exp
                    p, slot = h // 2, h % 2
                    lo, hi = CH * slot, CH * (slot + 1)
                    ktile, qtile = qk_tiles[2 * p], qk_tiles[2 * p + 1]
                    j, pl = st // 2, st % 2
                    pw = psB.tile([128, T], F32, tag="pw", name="pw")
                    for nb in range(NB):
                        nc.tensor.matmul(
                            pw[:, 512 * nb:512 * (nb + 1)],
                            ktile[lo:hi, 128 * st:128 * (st + 1)],
                            qtile[lo:hi, 512 * nb:512 * (nb + 1)],
                            start=True, stop=True)
                    if (h, j) not in ew_pairs:
                        ew_pairs[(h, j)] = ewp.tile([128, 2, T], FP8, tag="ew",
                                                    name=f"ew{h}_{j}")
                    nc.scalar.activation(ew_pairs[(h, j)][:, pl, :], pw[:], EXP)

                # vt pair tiles (fp8, rider block cols 0:RID with col0 = ones)
                vt_sb = [singles.tile([128, 2, H, RID + CH], FP8,
                                      tag=f"vt{j}", name=f"vt{j}")
                         for j in range(NJ)]
                for j in range(NJ):
                    nc.vector.memset(vt_sb[j][:, :, :, 0:RID], 0.0)
                    nc.vector.memset(vt_sb[j][:, :, :, 0:1], 1.0)

                def emit_v_tile(st):
                    pv = psB.tile([128, C], F32, tag="pw", name="pv")
                    for k in range(NCHUNK):
                        nc.tensor.matmul(pv[:],
                                         h_sb[k][:, 128 * st:128 * (st + 1)],
                                         wv_sb[k][:], start=(k == 0),
                                         stop=(k == 3))
                    nc.vector.tensor_copy(
                        vt_sb[st // 2][:, st % 2, :, RID:RID + CH],
                        pv[:].rearrange("p (h c) -> p h c", h=H))

                a_sb = [singles.tile([128, T], BF16, tag=f"a{p}",
                                     name=f"a{p}{sfx}") for p in range(NCHUNK)]
                acc_sb = [singles.tile([128, T], F32, tag=f"acc{m}",
                                       name=f"acc{m}{sfx}")
                          for m in range(NCHUNK)]

                # ---- prologue: QK(0) steps interleaved with v tiles ----
                for st in range(NT):
                    emit_qk_step(0, st)
                    emit_v_tile(st)

                # ================= duties =================
                def qk_spread_duty(m):
                    # produce qk tile m: 8 matmuls + 2 half bias-copies
                    pq = psA.tile([128, T], F32, tag="big", name="pq")
                    qk = singles.tile([128, T], BF16, tag=f"qk{m}{sfx}",
                                      name=f"qk{m}")
                    qk_tiles[m] = qk
                    for nb in range(NB):
                        sl = slice(512 * nb, 512 * (nb + 1))
                        for k in range(NCHUNK):
                            nc.tensor.matmul(
                                pq[:, sl], wqk_sb[k][:, 128 * m:128 * (m + 1)],
                                h_sb[k][:, sl], start=(k == 0), stop=(k == 3))
                            yield
                        nc.vector.tensor_scalar(
                            out=qk[:, sl], in0=pq[:, sl],
                            scalar1=bqk_sb[:, m:m + 1], scalar2=None,
                            op0=mybir.AluOpType.add)
                    yield

                def proj01_duty():
                    # acc[m] = (Wp0 @ a0 + Wp1 @ a1 + bproj) + x
                    for m in range(NCHUNK):
                        po = psA.tile([128, T], F32, tag="big", name="po")
                        for nb in range(NB):
                            sl = slice(512 * nb, 512 * (nb + 1))
                            nc.tensor.matmul(
                                po[:, sl], wproj_sb[0][:, 128 * m:128 * (m + 1)],
                                a_sb[0][:, sl], start=True, stop=False)
                            yield
                            nc.tensor.matmul(
                                po[:, sl], wproj_sb[1][:, 128 * m:128 * (m + 1)],
                                a_sb[1][:, sl], start=False, stop=True)
                            yield
                        nc.vector.scalar_tensor_tensor(
                            out=acc_sb[m][:], in0=po[:],
                            scalar=bproj_sb[:, m:m + 1], in1=x_sb[m][:],
                            op0=mybir.AluOpType.add, op1=mybir.AluOpType.add)
                        yield

                def proj_single_duty(p, crange):
                    # acc[m] += Wp[p][crange] @ a[p][crange]
                    clo, chi = crange
                    for m in range(NCHUNK):
                        po = psA.tile([128, T], F32, tag="big", name="po")
                        for nb in range(NB):
                            sl = slice(512 * nb, 512 * (nb + 1))
                            nc.tensor.matmul(
                                po[:, sl],
                                wproj_sb[p][clo:chi, 128 * m:128 * (m + 1)],
                                a_sb[p][clo:chi, sl], start=True, stop=True)
                            yield
                        nc.vector.tensor_add(acc_sb[m][:], po[:], acc_sb[m][:])
                        yield

                # ================= head loop =================
                for h in range(H):
                    p, slot = h // 2, h % 2
                    lo, hi = CH * slot, CH * (slot + 1)
                    duties = []
                    if h == 0:
                        duties.append(qk_spread_duty(2))
                        duties.append(qk_spread_duty(3))
                    if h == 1:
                        duties.append(qk_spread_duty(4))
                        duties.append(qk_spread_duty(5))
                    if h == 3:
                        duties.append(qk_spread_duty(6))
                        duties.append(qk_spread_duty(7))
                    if h == 5:
                        duties.append(proj01_duty())
                    if h == 6:
                        duties.append(proj_single_duty(2, (0, 128)))
                    if h == 7:
                        duties.append(proj_single_duty(3, (0, CH)))
                    pa = psA.tile([128, T], F32, tag="big", name="pa")
                    for st in range(NT):
                        if h + 1 < H:
                            emit_qk_step(h + 1, st)
                        advanced = 0
                        while duties and advanced < 3:
                            try:
                                next(duties[0])
                                advanced += 1
                            except StopIteration:
                                duties.pop(0)
                        if st % 2 == 1:
                            j = st // 2
                            ewt = ew_pairs.pop((h, j))
                            for nb in range(NB):
                                nc.tensor.matmul(
                                    pa[0:RID + CH, 512 * nb:512 * (nb + 1)],
                                    vt_sb[j][:, :, h, :],
                                    ewt[:, :, 512 * nb:512 * (nb + 1)],
                                    start=(j == 0), stop=(j == NJ - 1),
                                    perf_mode=DR)
                    for g in duties:
                        for _ in g:
                            pass
                    # ---- normalize: a_hat = pa[RID:] / pa[0] ----
                    if h == H - 1:
                        for nb in range(NB):
                            sl = slice(512 * nb, 512 * (nb + 1))
                            rcb = recp.tile([1, 512], F32, tag="rcb",
                                            name="rcb")
                            nc.vector.reciprocal_approx_fast(rcb[:], pa[0:1, sl])
                            rbb = rbp.tile([CH, 512], F32, tag="rbb",
                                           name="rbb")
                            nc.gpsimd.partition_broadcast(rbb[:], rcb[:])
                            nc.vector.tensor_mul(
                                a_sb[p][lo:hi, sl], pa[RID:RID + CH, sl], rbb[:])
                    else:
                        rec = recp.tile([1, T], F32, tag="rec", name="rec")
                        nc.vector.reciprocal_approx_fast(rec[:], pa[0:1, :])
                        rb = rbp.tile([CH, T], F32, tag="rb", name="rb")
                        nc.gpsimd.partition_broadcast(rb[:], rec[:])
                        nc.vector.tensor_mul(a_sb[p][lo:hi, :],
                                             pa[RID:RID + CH, :], rb[:])

                # ====== tail: pair-3 high half + out ======
                for m in range(NCHUNK):
                    po = psA.tile([128, T], F32, tag="big", name="po")
                    for nb in range(NB):
                        sl = slice(512 * nb, 512 * (nb + 1))
                        nc.tensor.matmul(
                            po[:, sl], wproj_sb[3][CH:128, 128 * m:128 * (m + 1)],
                            a_sb[3][CH:128, sl], start=True, stop=True)
                    for nb in range(NB):
                        sl = slice(512 * nb, 512 * (nb + 1))
                        nc.vector.tensor_add(acc_sb[m][:, sl], po[:, sl],
                                             acc_sb[m][:, sl])
                        if rep == n_reps - 1:
                            nc.sync.dma_start(out_d[128 * m:128 * (m + 1), sl],
                                              acc_sb[m][:, sl])

    nc.compile()
    return nc


def _get_program(n_reps=1):
    key = ("prog", n_reps)
    if key not in _CACHE:
        _CACHE[key] = _build_program(n_reps)
    return _CACHE[key]


def kernel(x, gn_w, gn_b, qkv_w, qkv_b, proj_w, proj_b, _n_reps=1):
    x = np.asarray(x, dtype=np.float32)
    hw = _host_weights(np.asarray(gn_w, np.float32), np.asarray(gn_b, np.float32),
                       np.asarray(qkv_w, np.float32), np.asarray(qkv_b, np.float32),
                       np.asarray(proj_w, np.float32), np.asarray(proj_b, np.float32))
    xr = np.ascontiguousarray(x.reshape(B, C, T))
    nc = _get_program(_n_reps)
    in_maps = [dict(hw, x=xr[b]) for b in range(B)]
    res = run_bass_kernel_spmd(nc, in_maps, core_ids=list(range(B)))
    out = np.stack([res.results[b]["out"] for b in range(B)])
    return out.reshape(B, C, HS, WS).astype(np.float32)


# revision 9
# speedup vs baseline: 1.1482x; 1.0912x over previous
"""Trainium2 Bass kernel for MultiHeadSelfAttention (GroupNorm + QKV + attention + proj + residual).

Problem shape (hardcoded): x [8, 512, 32, 32] fp32, 8 heads, 32 groups.
Sharding: data-parallel over batch B=8 across the 8 NeuronCores (one batch per core).

Per-core pipeline (T = 1024 positions, C = 512 channels, ch = 64 per head):
  1. GroupNorm(32) chunk-pipelined: groups never cross a 128-channel chunk;
     rsqrt(var+eps) via the quake bit-hack + 2 Newton steps on DVE so the
     Activation engine runs softmax exps only (no act-table swaps).
  2. qkv = qkv_w @ h with host-reordered bf16 weights:
       - q,k tiles [128, T]: m-tile 2p = [k_h(2p)|k_h(2p+1)], 2p+1 = [q...]
       - v produced transposed per s-tile, packed as fp8e4 pairs for DoubleRow
  3. Per head: logits via PE (bf16), ONE merged exp [128,1024] per s-tile on
     ACT writing fp8e4 straight to SBUF; attention @ V via fp8 DoubleRow
     matmuls (two s-planes per instruction; 64-col rider block carries the
     softmax denominator in partition 0); DVE reciprocal + gpsimd
     partition_broadcast + DVE mul to normalize.
  4. proj accumulated in PSUM per pair-group; v-bias folded into the proj
     bias on the host; bias+residual fused via scalar_tensor_tensor. Only
     the head-7 contraction half remains for the tail.

All input DMAs ride one ordered SP queue (x + wqk first) so the first
softmax exp lands as early as possible; the exp stream is the critical
resource and runs back-to-back for the rest of the kernel.
"""

import ml_dtypes
import numpy as np

import concourse.bass as bass
import concourse.bacc as bacc
import concourse.tile as tile
import concourse.mybir as mybir
from concourse import library_config
from concourse.bass_utils import run_bass_kernel_spmd

B, C, HS, WS = 8, 512, 32, 32
T = HS * WS            # 1024
H = 8                  # heads
CH = C // H            # 64
G = 32                 # groups
CPG = C // G           # 16 channels per group
EPS = 1e-5
NCHUNK = C // 128      # 4 channel chunks
NT = T // 128          # 8 sequence tiles
NB = T // 512          # 2 psum banks over T
NJ = NT // 2           # 4 s-tile pairs (DoubleRow planes)
RID = 64               # rider cols per head (col 0 = ones); out partitions 128
MAGIC = 0x5F3759DF     # quake rsqrt seed
F32 = mybir.dt.float32
F32R = mybir.dt.float32r
I32 = mybir.dt.int32
BF16 = mybir.dt.bfloat16
FP8 = mybir.dt.float8e4
EXP = mybir.ActivationFunctionType.Exp
IDENT = mybir.ActivationFunctionType.Identity
DR = mybir.MatmulPerfMode.DoubleRow
MUL = mybir.AluOpType.mult
ADD = mybir.AluOpType.add
SHR = mybir.AluOpType.logical_shift_right

_CACHE = {}


def _orig_row(kind, h, i):
    off = {"q": 0, "k": CH, "v": 2 * CH}[kind]
    return 192 * h + off + i


def _host_weights(gn_w, gn_b, qkv_w, qkv_b, proj_w, proj_b):
    scale2 = 1.0 / np.sqrt(CH)  # ch**-0.25 on both q and k -> fold into k
    rows = np.zeros(2 * C, dtype=np.int64)
    colscale = np.ones(2 * C, dtype=np.float32)
    for p in range(H // 2):
        for slot in range(2):
            h = 2 * p + slot
            for i in range(CH):
                col_k = (2 * p) * 128 + slot * CH + i
                rows[col_k] = _orig_row("k", h, i)
                colscale[col_k] = scale2
                col_q = (2 * p + 1) * 128 + slot * CH + i
                rows[col_q] = _orig_row("q", h, i)
    wqk = (qkv_w[rows, :] * colscale[:, None]).T.copy()      # [512, 1024]
    # two DMA tiles: chunks (0,1) and (2,3) side by side
    wqk_t = np.ascontiguousarray(
        wqk.reshape(2, 2, 128, 2 * C).transpose(0, 2, 1, 3).reshape(
            2, 128, 4 * C)).astype(ml_dtypes.bfloat16)
    bqk = (qkv_b[rows] * colscale).reshape(8, 128).T.copy()  # [128, 8]

    vrows = np.array([_orig_row("v", h, i) for h in range(H) for i in range(CH)])
    wv = qkv_w[vrows, :].T.copy()                            # [512, 512] (c, c_v)
    wv_t = np.ascontiguousarray(
        wv.reshape(NCHUNK, 128, C).transpose(1, 0, 2).reshape(
            128, NCHUNK * C)).astype(ml_dtypes.bfloat16)     # [128, 2048]

    bv = qkv_b[vrows]
    bproj_full = proj_b + proj_w @ bv                        # [512]
    wproj = proj_w.T.copy()                                  # [512(c), 512(o)]
    wproj_t = np.ascontiguousarray(
        wproj.reshape(NCHUNK, 128, C).transpose(1, 0, 2).reshape(
            128, NCHUNK * C)).astype(ml_dtypes.bfloat16)

    # consolidated f32 consts [128, 24]: g8 | gnw | gnb | bqk | bproj
    g8 = np.zeros((128, 8), dtype=np.float32)
    gt8 = np.zeros((8, 128), dtype=np.float32)
    for u in range(128):
        g8[u, u // CPG] = 1.0 / CPG
        gt8[u // CPG, u] = 1.0
    cst = np.concatenate([
        g8,
        gn_w.reshape(NCHUNK, 128).T,
        gn_b.reshape(NCHUNK, 128).T,
        bqk,
        bproj_full.reshape(NCHUNK, 128).T,
    ], axis=1).astype(np.float32)                            # [128, 28]
    return {"cst": cst, "gt8": gt8, "wqk": wqk_t, "wv": wv_t,
            "wproj": wproj_t}


def _build_program(n_reps=1, ew_bufs=12):
    nc = bacc.Bacc("TRN2", target_bir_lowering=False, debug=False, num_devices=8)
    dt_in = [
        ("x", [C, T], F32), ("cst", [128, 28], F32R), ("gt8", [8, 128], F32R),
        ("wqk", [2, 128, 4 * C], BF16), ("wv", [128, NCHUNK * C], BF16),
        ("wproj", [128, NCHUNK * C], BF16),
    ]
    d = {name: nc.dram_tensor(name, shape, dt, kind="ExternalInput").ap()
         for name, shape, dt in dt_in}
    out_d = nc.dram_tensor("out", [C, T], F32, kind="ExternalOutput").ap()

    with tile.TileContext(nc) as tc:
        with (
            tc.tile_pool(name="singles", bufs=1) as singles,
            tc.tile_pool(name="small", bufs=16) as small,
            tc.tile_pool(name="ewp", bufs=ew_bufs) as ewp,
            tc.tile_pool(name="recp", bufs=2) as recp,
            tc.tile_pool(name="rbp", bufs=2) as rbp,
            tc.tile_pool(name="psA", bufs=2, space="PSUM") as psA,
            tc.tile_pool(name="psB", bufs=2, space="PSUM") as psB,
        ):
            nc.gpsimd.load_library(library_config.attn)

            # ---- one ordered DMA stream on the SP queue: consts, then x
            # ---- halves interleaved with wqk, then wv/wproj ----
            cst = singles.tile([128, 28], F32R, tag="cst", name="cst")
            nc.sync.dma_start(cst[:], d["cst"][:])
            gt8_sb = singles.tile([8, 128], F32R, tag="gt8", name="gt8")
            nc.sync.dma_start(gt8_sb[:], d["gt8"][:])
            g8_sb = cst[:, 0:8]
            gnw_sb = cst[:, 8:12].bitcast(F32)
            gnb_sb = cst[:, 12:16].bitcast(F32)
            bqk_sb = cst[:, 16:24].bitcast(F32)
            bproj_sb = cst[:, 24:28].bitcast(F32)

            x_sb = [singles.tile([128, T], F32, tag=f"x{k}", name=f"x{k}")
                    for k in range(NCHUNK)]
            wqk_sb = [singles.tile([128, 4 * C], BF16, tag=f"wqk{g}",
                                   name=f"wqk{g}") for g in range(2)]
            for k in range(NCHUNK):
                for nb in range(NB):
                    sl = slice(512 * nb, 512 * (nb + 1))
                    nc.sync.dma_start(x_sb[k][:, sl],
                                      d["x"][128 * k:128 * (k + 1), sl])
                if k == 1:
                    nc.sync.dma_start(wqk_sb[0][:], d["wqk"][0])
                if k == 3:
                    nc.sync.dma_start(wqk_sb[1][:], d["wqk"][1])
            wv_sb = singles.tile([128, NCHUNK * C], BF16, tag="wv", name="wv")
            nc.sync.dma_start(wv_sb[:], d["wv"][:])
            wproj_sb = singles.tile([128, NCHUNK * C], BF16, tag="wproj",
                                    name="wproj")
            nc.sync.dma_start(wproj_sb[:], d["wproj"][:])

            def wqk_ap(k, m):
                # chunk k, m-tile column block [128, 128]
                return wqk_sb[k // 2][:, 1024 * (k % 2) + 128 * m:
                                      1024 * (k % 2) + 128 * (m + 1)]

            def wv_ap(k):
                return wv_sb[:, 512 * k:512 * (k + 1)]

            def wproj_ap(p, m, clo=0, chi=128):
                return wproj_sb[clo:chi, 512 * p + 128 * m:512 * p + 128 * (m + 1)]

            magic_t = singles.tile([8, 1], I32, tag="magic", name="magic")
            nc.vector.memset(magic_t[:], MAGIC)

            for rep in range(n_reps):
                sfx = f"r{rep}"
                # ================= GroupNorm (per chunk) =================
                h_sb = []
                for k in range(NCHUNK):
                    st6 = small.tile([128, 2, 6], F32, tag="small", name="st6")
                    nc.vector.bn_stats(st6[:, 0, :], x_sb[k][:, 0:512])
                    nc.vector.bn_stats(st6[:, 1, :], x_sb[k][:, 512:1024])
                    mv = small.tile([128, 2], F32, tag="small", name="mv")
                    nc.vector.bn_aggr(mv[:], st6[:])
                    m2 = small.tile([128, 1], F32, tag="small", name="m2")
                    nc.vector.tensor_mul(m2[:], mv[:, 0:1], mv[:, 0:1])
                    stats = small.tile([128, 2], F32R, tag="small", name="stats")
                    nc.vector.tensor_copy(stats[:, 0:1], mv[:, 0:1])
                    nc.vector.tensor_add(stats[:, 1:2], mv[:, 1:2], m2[:])
                    psg = psA.tile([8, 2], F32, tag="big", name="psg")
                    nc.tensor.matmul(psg[:], g8_sb, stats[:],
                                     start=True, stop=True)
                    gsb = small.tile([8, 2], F32, tag="small", name="gsb")
                    nc.vector.tensor_copy(gsb[:], psg[:])
                    mu2 = small.tile([8, 1], F32, tag="small", name="mu2")
                    nc.vector.tensor_mul(mu2[:], gsb[:, 0:1], gsb[:, 0:1])
                    # a = var + eps ;  rstd = rsqrt(a) via bit hack + 2 Newton
                    av = small.tile([8, 1], F32, tag="small", name="av")
                    nc.vector.tensor_sub(av[:], gsb[:, 1:2], mu2[:])
                    nc.vector.tensor_scalar(out=av[:], in0=av[:], scalar1=EPS,
                                            scalar2=None, op0=ADD)
                    yi = small.tile([8, 1], I32, tag="small", name="yi")
                    nc.vector.tensor_scalar(out=yi[:], in0=av[:].bitcast(I32),
                                            scalar1=1, scalar2=None, op0=SHR)
                    nc.vector.tensor_sub(yi[:], magic_t[:], yi[:])
                    y = yi[:].bitcast(F32)
                    ah = small.tile([8, 1], F32, tag="small", name="ah")
                    nc.vector.tensor_scalar(out=ah[:], in0=av[:], scalar1=0.5,
                                            scalar2=None, op0=MUL)
                    t2 = small.tile([8, 1], F32, tag="small", name="t2")
                    for _ in range(2):
                        nc.vector.tensor_mul(t2[:], y, y)
                        nc.vector.tensor_mul(t2[:], t2[:], ah[:])
                        nc.vector.tensor_scalar(out=t2[:], in0=t2[:],
                                                scalar1=-1.0, scalar2=1.5,
                                                op0=MUL, op1=ADD)
                        nc.vector.tensor_mul(y, y, t2[:])
                    grp = small.tile([8, 2], F32R, tag="small", name="grp")
                    nc.vector.tensor_copy(grp[:, 0:1], gsb[:, 0:1])
                    nc.vector.tensor_copy(grp[:, 1:2], y)
                    psc = psA.tile([128, 2], F32, tag="big", name="psc")
                    nc.tensor.matmul(psc[:], gt8_sb[:], grp[:],
                                     start=True, stop=True)
                    s_c = small.tile([128, 1], F32, tag="small", name="s_c")
                    nc.vector.tensor_mul(s_c[:], psc[:, 1:2], gnw_sb[:, k:k + 1])
                    t1 = small.tile([128, 1], F32, tag="small", name="t1")
                    nc.vector.tensor_mul(t1[:], psc[:, 0:1], s_c[:])
                    b_c = small.tile([128, 1], F32, tag="small", name="b_c")
                    nc.vector.tensor_sub(b_c[:], gnb_sb[:, k:k + 1], t1[:])
                    ht = singles.tile([128, T], BF16, tag=f"h{k}", name=f"h{k}")
                    for nb in range(NB):
                        sl = slice(512 * nb, 512 * (nb + 1))
                        nc.vector.tensor_scalar(
                            out=ht[:, sl], in0=x_sb[k][:, sl], scalar1=s_c[:],
                            scalar2=b_c[:], op0=MUL, op1=ADD)
                    h_sb.append(ht)

                # ================= qk tiles =================
                qk_tiles = {}

                def gen_qk(m, act_copy=False):
                    pq = psA.tile([128, T], F32, tag="big", name="pq")
                    qk = singles.tile([128, T], BF16, tag=f"qk{m}{sfx}",
                                      name=f"qk{m}")
                    qk_tiles[m] = qk
                    for nb in range(NB):
                        sl = slice(512 * nb, 512 * (nb + 1))
                        for k in range(NCHUNK):
                            nc.tensor.matmul(
                                pq[:, sl], wqk_ap(k, m), h_sb[k][:, sl],
                                start=(k == 0), stop=(k == 3))
                        if act_copy:
                            nc.scalar.activation(qk[:, sl], pq[:, sl], IDENT,
                                                 bias=bqk_sb[:, m:m + 1])
                        else:
                            nc.vector.tensor_scalar(
                                out=qk[:, sl], in0=pq[:, sl],
                                scalar1=bqk_sb[:, m:m + 1], scalar2=None,
                                op0=ADD)

                gen_qk(0, act_copy=True)
                gen_qk(1, act_copy=False)

                # ================= attention state =================
                ew_pairs = {}

                def emit_qk_step(h, st):
                    # logits for head h, s-tile st: 2 matmuls + 1 merged exp
                    p, slot = h // 2, h % 2
                    lo, hi = CH * slot, CH * (slot + 1)
                    ktile, qtile = qk_tiles[2 * p], qk_tiles[2 * p + 1]
                    j, pl = st // 2, st % 2
                    pw = psB.tile([128, T], F32, tag="pw", name="pw")
                    for nb in range(NB):
                        nc.tensor.matmul(
                            pw[:, 512 * nb:512 * (nb + 1)],
                            ktile[lo:hi, 128 * st:128 * (st + 1)],
                            qtile[lo:hi, 512 * nb:512 * (nb + 1)],
                            start=True, stop=True)
                    if (h, j) not in ew_pairs:
                        ew_pairs[(h, j)] = ewp.tile([128, 2, T], FP8, tag="ew",
                                                    name=f"ew{h}_{j}")
                    nc.scalar.activation(ew_pairs[(h, j)][:, pl, :], pw[:], EXP)

                # vt pair tiles (fp8, rider block cols 0:RID with col0 = ones)
                vt_sb = [singles.tile([128, 2, H, RID + CH], FP8,
                                      tag=f"vt{j}", name=f"vt{j}")
                         for j in range(NJ)]
                for j in range(NJ):
                    nc.vector.memset(vt_sb[j][:, :, :, 0:RID], 0.0)
                    nc.vector.memset(vt_sb[j][:, :, :, 0:1], 1.0)

                def emit_v_tile(st):
                    pv = psA.tile([128, C], F32, tag="big", name="pv")
                    for k in range(NCHUNK):
                        nc.tensor.matmul(pv[:],
                                         h_sb[k][:, 128 * st:128 * (st + 1)],
                                         wv_ap(k), start=(k == 0),
                                         stop=(k == 3))
                    nc.vector.tensor_copy(
                        vt_sb[st // 2][:, st % 2, :, RID:RID + CH],
                        pv[:].rearrange("p (h c) -> p h c", h=H))

                a_sb = [singles.tile([128, T], BF16, tag=f"a{p}",
                                     name=f"a{p}{sfx}") for p in range(NCHUNK)]
                acc_sb = [singles.tile([128, T], F32, tag=f"acc{m}",
                                       name=f"acc{m}{sfx}")
                          for m in range(NCHUNK)]

                # ---- prologue: v tiles + QK(0) steps (pv uses the big tag
                # ---- so the pw rotation stays a pure QK/exp double-buffer)
                for st in range(NT):
                    emit_qk_step(0, st)
                    emit_v_tile(st)

                # ================= duties =================
                def qk_spread_duty(m):
                    pq = psA.tile([128, T], F32, tag="big", name="pq")
                    qk = singles.tile([128, T], BF16, tag=f"qk{m}{sfx}",
                                      name=f"qk{m}")
                    qk_tiles[m] = qk
                    for nb in range(NB):
                        sl = slice(512 * nb, 512 * (nb + 1))
                        for k in range(NCHUNK):
                            nc.tensor.matmul(
                                pq[:, sl], wqk_ap(k, m), h_sb[k][:, sl],
                                start=(k == 0), stop=(k == 3))
                            yield
                        nc.vector.tensor_scalar(
                            out=qk[:, sl], in0=pq[:, sl],
                            scalar1=bqk_sb[:, m:m + 1], scalar2=None, op0=ADD)
                    yield

                def proj01_duty():
                    # acc[m] = (Wp0 @ a0 + Wp1 @ a1 + bproj) + x
                    for m in range(NCHUNK):
                        po = psA.tile([128, T], F32, tag="big", name="po")
                        for nb in range(NB):
                            sl = slice(512 * nb, 512 * (nb + 1))
                            nc.tensor.matmul(
                                po[:, sl], wproj_ap(0, m), a_sb[0][:, sl],
                                start=True, stop=False)
                            yield
                            nc.tensor.matmul(
                                po[:, sl], wproj_ap(1, m), a_sb[1][:, sl],
                                start=False, stop=True)
                            yield
                        nc.vector.scalar_tensor_tensor(
                            out=acc_sb[m][:], in0=po[:],
                            scalar=bproj_sb[:, m:m + 1], in1=x_sb[m][:],
                            op0=ADD, op1=ADD)
                        yield

                def proj23lo_duty():
                    # acc[m] += Wp2 @ a2 + Wp3[0:64] @ a3[0:64]
                    for m in range(NCHUNK):
                        po = psA.tile([128, T], F32, tag="big", name="po")
                        for nb in range(NB):
                            sl = slice(512 * nb, 512 * (nb + 1))
                            nc.tensor.matmul(
                                po[:, sl], wproj_ap(2, m), a_sb[2][:, sl],
                                start=True, stop=False)
                            yield
                            nc.tensor.matmul(
                                po[:, sl], wproj_ap(3, m, 0, CH),
                                a_sb[3][0:CH, sl], start=False, stop=True)
                            yield
                        nc.vector.tensor_add(acc_sb[m][:], po[:], acc_sb[m][:])
                        yield

                # ================= head loop =================
                for h in range(H):
                    p, slot = h // 2, h % 2
                    lo, hi = CH * slot, CH * (slot + 1)
                    duties = []
                    if h == 0:
                        duties.append(qk_spread_duty(2))
                        duties.append(qk_spread_duty(3))
                    if h == 1:
                        duties.append(qk_spread_duty(4))
                        duties.append(qk_spread_duty(5))
                    if h == 3:
                        duties.append(qk_spread_duty(6))
                        duties.append(qk_spread_duty(7))
                    if h == 5:
                        duties.append(proj01_duty())
                    if h == 7:
                        duties.append(proj23lo_duty())
                    pa = psA.tile([128, T], F32, tag="big", name="pa")
                    for st in range(NT):
                        if h + 1 < H:
                            emit_qk_step(h + 1, st)
                        advanced = 0
                        while duties and advanced < 3:
                            try:
                                next(duties[0])
                                advanced += 1
                            except StopIteration:
                                duties.pop(0)
                        if st % 2 == 1:
                            j = st // 2
                            ewt = ew_pairs.pop((h, j))
                            for nb in range(NB):
                                nc.tensor.matmul(
                                    pa[:, 512 * nb:512 * (nb + 1)],
                                    vt_sb[j][:, :, h, :],
                                    ewt[:, :, 512 * nb:512 * (nb + 1)],
                                    start=(j == 0), stop=(j == NJ - 1),
                                    perf_mode=DR)
                    for g in duties:
                        for _ in g:
                            pass
                    # ---- normalize: a_hat = pa[RID:] / pa[0] ----
                    if h == H - 1:
                        for nb in range(NB):
                            sl = slice(512 * nb, 512 * (nb + 1))
                            rcb = recp.tile([1, 512], F32, tag="rcb",
                                            name="rcb")
                            nc.vector.reciprocal_approx_fast(rcb[:], pa[0:1, sl])
                            rbb = rbp.tile([CH, 512], F32, tag="rbb",
                                           name="rbb")
                            nc.gpsimd.partition_broadcast(rbb[:], rcb[:])
                            nc.vector.tensor_mul(
                                a_sb[p][lo:hi, sl], pa[RID:RID + CH, sl], rbb[:])
                    else:
                        rec = recp.tile([1, T], F32, tag="rec", name="rec")
                        nc.vector.reciprocal_approx_fast(rec[:], pa[0:1, :])
                        rb = rbp.tile([CH, T], F32, tag="rb", name="rb")
                        nc.gpsimd.partition_broadcast(rb[:], rec[:])
                        nc.vector.tensor_mul(a_sb[p][lo:hi, :],
                                             pa[RID:RID + CH, :], rb[:])

                # ====== tail: pair-3 high half + out, half-tile staggered ======
                po_t = {}
                for m in range(NCHUNK):
                    po_t[m] = psA.tile([128, T], F32, tag="big", name="po")
                    for nb in range(NB):
                        sl = slice(512 * nb, 512 * (nb + 1))
                        nc.tensor.matmul(
                            po_t[m][:, sl], wproj_ap(3, m, CH, 128),
                            a_sb[3][CH:128, sl], start=True, stop=True)
                    for nb in range(NB):
                        sl = slice(512 * nb, 512 * (nb + 1))
                        nc.vector.tensor_add(acc_sb[m][:, sl], po_t[m][:, sl],
                                             acc_sb[m][:, sl])
                        if rep == n_reps - 1:
                            nc.sync.dma_start(out_d[128 * m:128 * (m + 1), sl],
                                              acc_sb[m][:, sl])

    nc.compile()
    return nc


def _get_program(n_reps=1):
    key = ("prog", n_reps)
    if key not in _CACHE:
        _CACHE[key] = _build_program(n_reps)
    return _CACHE[key]


def kernel(x, gn_w, gn_b, qkv_w, qkv_b, proj_w, proj_b, _n_reps=1):
    x = np.asarray(x, dtype=np.float32)
    hw = _host_weights(np.asarray(gn_w, np.float32), np.asarray(gn_b, np.float32),
                       np.asarray(qkv_w, np.float32), np.asarray(qkv_b, np.float32),
                       np.asarray(proj_w, np.float32), np.asarray(proj_b, np.float32))
    xr = np.ascontiguousarray(x.reshape(B, C, T))
    nc = _get_program(_n_reps)
    in_maps = [dict(hw, x=xr[b]) for b in range(B)]
    res = run_bass_kernel_spmd(nc, in_maps, core_ids=list(range(B)))
    out = np.stack([res.results[b]["out"] for b in range(B)])
    return out.reshape(B, C, HS, WS).astype(np.float32)


# revision 11
# speedup vs baseline: 1.1957x; 1.0414x over previous
"""Trainium2 Bass kernel for MultiHeadSelfAttention (GroupNorm + QKV + attention + proj + residual).

Problem shape (hardcoded): x [8, 512, 32, 32] fp32, 8 heads, 32 groups.
Sharding: data-parallel over batch B=8 across the 8 NeuronCores (one batch per core).

Per-core pipeline (T = 1024 positions, C = 512 channels, ch = 64 per head):
  1. GroupNorm(32) chunk-pipelined: groups never cross a 128-channel chunk;
     rsqrt(var+eps) via the quake bit-hack + 2 Newton steps on DVE so the
     Activation engine runs softmax exps only (no act-table swaps).
  2. qkv = qkv_w @ h with host-reordered bf16 weights:
       - q,k tiles [128, T]: m-tile 2p = [k_h(2p)|k_h(2p+1)], 2p+1 = [q...]
       - v produced transposed per s-tile, packed as fp8e4 pairs for DoubleRow
  3. Per head: logits via PE (bf16), ONE merged exp [128,1024] per s-tile on
     ACT writing fp8e4 straight to SBUF; attention @ V via fp8 DoubleRow
     matmuls (two s-planes per instruction; 64-col rider block carries the
     softmax denominator in partition 0); DVE reciprocal + gpsimd
     partition_broadcast + DVE mul to normalize.
  4. proj accumulated in PSUM per pair-group; v-bias folded into the proj
     bias on the host; bias+residual fused via scalar_tensor_tensor. Only
     the head-7 contraction half remains for the tail.

All input DMAs ride one ordered SP queue (x + wqk first) so the first
softmax exp lands as early as possible; the exp stream is the critical
resource and runs back-to-back for the rest of the kernel.
"""

import ml_dtypes
import numpy as np

import concourse.bass as bass
import concourse.bacc as bacc
import concourse.tile as tile
import concourse.mybir as mybir
from concourse import library_config
from concourse.bass_utils import run_bass_kernel_spmd

B, C, HS, WS = 8, 512, 32, 32
T = HS * WS            # 1024
H = 8                  # heads
CH = C // H            # 64
G = 32                 # groups
CPG = C // G           # 16 channels per group
EPS = 1e-5
NCHUNK = C // 128      # 4 channel chunks
NT = T // 128          # 8 sequence tiles
NB = T // 512          # 2 psum banks over T
NJ = NT // 2           # 4 s-tile pairs (DoubleRow planes)
RID = 64               # rider cols per head (col 0 = ones); out partitions 128
MAGIC = 0x5F3759DF     # quake rsqrt seed
F32 = mybir.dt.float32
F32R = mybir.dt.float32r
I32 = mybir.dt.int32
BF16 = mybir.dt.bfloat16
FP8 = mybir.dt.float8e4
EXP = mybir.ActivationFunctionType.Exp
IDENT = mybir.ActivationFunctionType.Identity
DR = mybir.MatmulPerfMode.DoubleRow
MUL = mybir.AluOpType.mult
ADD = mybir.AluOpType.add
SHR = mybir.AluOpType.logical_shift_right

_CACHE = {}


def _orig_row(kind, h, i):
    off = {"q": 0, "k": CH, "v": 2 * CH}[kind]
    return 192 * h + off + i


def _host_weights(gn_w, gn_b, qkv_w, qkv_b, proj_w, proj_b):
    scale2 = 1.0 / np.sqrt(CH)  # ch**-0.25 on both q and k -> fold into k
    rows = np.zeros(2 * C, dtype=np.int64)
    colscale = np.ones(2 * C, dtype=np.float32)
    for p in range(H // 2):
        for slot in range(2):
            h = 2 * p + slot
            for i in range(CH):
                col_k = (2 * p) * 128 + slot * CH + i
                rows[col_k] = _orig_row("k", h, i)
                colscale[col_k] = scale2
                col_q = (2 * p + 1) * 128 + slot * CH + i
                rows[col_q] = _orig_row("q", h, i)
    wqk = (qkv_w[rows, :] * colscale[:, None]).T.copy()      # [512, 1024]
    # two DMA tiles: chunks (0,1) and (2,3) side by side
    wqk_t = np.ascontiguousarray(
        wqk.reshape(2, 2, 128, 2 * C).transpose(0, 2, 1, 3).reshape(
            2, 128, 4 * C)).astype(ml_dtypes.bfloat16)
    bqk = (qkv_b[rows] * colscale).reshape(8, 128).T.copy()  # [128, 8]

    vrows = np.array([_orig_row("v", h, i) for h in range(H) for i in range(CH)])
    wv = qkv_w[vrows, :].T.copy()                            # [512, 512] (c, c_v)
    wv_t = np.ascontiguousarray(
        wv.reshape(NCHUNK, 128, C).transpose(1, 0, 2).reshape(
            128, NCHUNK * C)).astype(ml_dtypes.bfloat16)     # [128, 2048]

    bv = qkv_b[vrows]
    bproj_full = proj_b + proj_w @ bv                        # [512]
    wproj = proj_w.T.copy()                                  # [512(c), 512(o)]
    wproj_t = np.ascontiguousarray(
        wproj.reshape(NCHUNK, 128, C).transpose(1, 0, 2).reshape(
            128, NCHUNK * C)).astype(ml_dtypes.bfloat16)

    # consolidated f32 consts [128, 24]: g8 | gnw | gnb | bqk | bproj
    g8 = np.zeros((128, 8), dtype=np.float32)
    gt8 = np.zeros((8, 128), dtype=np.float32)
    for u in range(128):
        g8[u, u // CPG] = 1.0 / CPG
        gt8[u // CPG, u] = 1.0
    cst = np.concatenate([
        g8,
        gn_w.reshape(NCHUNK, 128).T,
        gn_b.reshape(NCHUNK, 128).T,
        bqk,
        bproj_full.reshape(NCHUNK, 128).T,
    ], axis=1).astype(np.float32)                            # [128, 28]
    return {"cst": cst, "gt8": gt8, "wqk": wqk_t, "wv": wv_t,
            "wproj": wproj_t}


def _build_program(n_reps=1, ew_bufs=12):
    nc = bacc.Bacc("TRN2", target_bir_lowering=False, debug=False, num_devices=8)
    dt_in = [
        ("x", [C, T], F32), ("cst", [128, 28], F32R), ("gt8", [8, 128], F32R),
        ("wqk", [2, 128, 4 * C], BF16), ("wv", [128, NCHUNK * C], BF16),
        ("wproj", [128, NCHUNK * C], BF16),
    ]
    d = {name: nc.dram_tensor(name, shape, dt, kind="ExternalInput").ap()
         for name, shape, dt in dt_in}
    out_d = nc.dram_tensor("out", [C, T], F32, kind="ExternalOutput").ap()

    with tile.TileContext(nc) as tc:
        with (
            tc.tile_pool(name="singles", bufs=1) as singles,
            tc.tile_pool(name="small", bufs=16) as small,
            tc.tile_pool(name="ewp", bufs=ew_bufs) as ewp,
            tc.tile_pool(name="recp", bufs=2) as recp,
            tc.tile_pool(name="rbp", bufs=2) as rbp,
            tc.tile_pool(name="psA", bufs=2, space="PSUM") as psA,
            tc.tile_pool(name="psB", bufs=2, space="PSUM") as psB,
        ):
            nc.gpsimd.load_library(library_config.attn)

            # ---- one ordered DMA stream on the SP queue: consts, then x
            # ---- halves interleaved with wqk, then wv/wproj ----
            cst = singles.tile([128, 28], F32R, tag="cst", name="cst")
            nc.sync.dma_start(cst[:], d["cst"][:])
            gt8_sb = singles.tile([8, 128], F32R, tag="gt8", name="gt8")
            nc.sync.dma_start(gt8_sb[:], d["gt8"][:])
            g8_sb = cst[:, 0:8]
            gnw_sb = cst[:, 8:12].bitcast(F32)
            gnb_sb = cst[:, 12:16].bitcast(F32)
            bqk_sb = cst[:, 16:24].bitcast(F32)
            bproj_sb = cst[:, 24:28].bitcast(F32)

            x_sb = [singles.tile([128, T], F32, tag=f"x{k}", name=f"x{k}")
                    for k in range(NCHUNK)]
            wqk_sb = [singles.tile([128, 4 * C], BF16, tag=f"wqk{g}",
                                   name=f"wqk{g}") for g in range(2)]
            for k in range(NCHUNK):
                for nb in range(NB):
                    sl = slice(512 * nb, 512 * (nb + 1))
                    nc.sync.dma_start(x_sb[k][:, sl],
                                      d["x"][128 * k:128 * (k + 1), sl])
                if k == 1:
                    nc.sync.dma_start(wqk_sb[0][:], d["wqk"][0])
                if k == 3:
                    nc.sync.dma_start(wqk_sb[1][:], d["wqk"][1])
            wv_sb = singles.tile([128, NCHUNK * C], BF16, tag="wv", name="wv")
            nc.sync.dma_start(wv_sb[:], d["wv"][:])
            wproj_sb = singles.tile([128, NCHUNK * C], BF16, tag="wproj",
                                    name="wproj")
            nc.sync.dma_start(wproj_sb[:], d["wproj"][:])

            def wqk_ap(k, m):
                # chunk k, m-tile column block [128, 128]
                return wqk_sb[k // 2][:, 1024 * (k % 2) + 128 * m:
                                      1024 * (k % 2) + 128 * (m + 1)]

            def wv_ap(k):
                return wv_sb[:, 512 * k:512 * (k + 1)]

            def wproj_ap(p, m, clo=0, chi=128):
                return wproj_sb[clo:chi, 512 * p + 128 * m:512 * p + 128 * (m + 1)]

            magic_t = singles.tile([8, 1], I32, tag="magic", name="magic")
            nc.vector.memset(magic_t[:], MAGIC)

            for rep in range(n_reps):
                sfx = f"r{rep}"
                # ================= GroupNorm (per chunk) =================
                h_sb = []
                for k in range(NCHUNK):
                    # per-channel sum(x) and sum(x^2) on ACT (idle at startup)
                    scr = small.tile([128, T], BF16, tag="gnscr", bufs=2,
                                     name="scr")
                    asm = small.tile([128, 1], F32, tag="small", name="asm")
                    nc.scalar.activation(scr[:], x_sb[k][:], IDENT,
                                         accum_out=asm[:])
                    asq = small.tile([128, 1], F32, tag="small", name="asq")
                    nc.scalar.activation(scr[:], x_sb[k][:],
                                         mybir.ActivationFunctionType.Square,
                                         accum_out=asq[:])
                    stats = small.tile([128, 2], F32R, tag="small", name="stats")
                    nc.vector.tensor_scalar(out=stats[:, 0:1], in0=asm[:],
                                            scalar1=1.0 / T, scalar2=None,
                                            op0=MUL)
                    nc.vector.tensor_scalar(out=stats[:, 1:2], in0=asq[:],
                                            scalar1=1.0 / T, scalar2=None,
                                            op0=MUL)
                    psg = psA.tile([8, 2], F32, tag="big", name="psg")
                    nc.tensor.matmul(psg[:], g8_sb, stats[:],
                                     start=True, stop=True)
                    gsb = small.tile([8, 2], F32, tag="small", name="gsb")
                    nc.vector.tensor_copy(gsb[:], psg[:])
                    mu2 = small.tile([8, 1], F32, tag="small", name="mu2")
                    nc.vector.tensor_mul(mu2[:], gsb[:, 0:1], gsb[:, 0:1])
                    # a = var + eps ;  rstd = rsqrt(a) via bit hack + 2 Newton
                    av = small.tile([8, 1], F32, tag="small", name="av")
                    nc.vector.tensor_sub(av[:], gsb[:, 1:2], mu2[:])
                    nc.vector.tensor_scalar(out=av[:], in0=av[:], scalar1=EPS,
                                            scalar2=None, op0=ADD)
                    yi = small.tile([8, 1], I32, tag="small", name="yi")
                    nc.vector.tensor_scalar(out=yi[:], in0=av[:].bitcast(I32),
                                            scalar1=1, scalar2=None, op0=SHR)
                    nc.vector.tensor_sub(yi[:], magic_t[:], yi[:])
                    y = yi[:].bitcast(F32)
                    ah = small.tile([8, 1], F32, tag="small", name="ah")
                    nc.vector.tensor_scalar(out=ah[:], in0=av[:], scalar1=0.5,
                                            scalar2=None, op0=MUL)
                    t2 = small.tile([8, 1], F32, tag="small", name="t2")
                    for _ in range(2):
                        nc.vector.tensor_mul(t2[:], y, y)
                        nc.vector.tensor_mul(t2[:], t2[:], ah[:])
                        nc.vector.tensor_scalar(out=t2[:], in0=t2[:],
                                                scalar1=-1.0, scalar2=1.5,
                                                op0=MUL, op1=ADD)
                        nc.vector.tensor_mul(y, y, t2[:])
                    grp = small.tile([8, 2], F32R, tag="small", name="grp")
                    nc.vector.tensor_copy(grp[:, 0:1], gsb[:, 0:1])
                    nc.vector.tensor_copy(grp[:, 1:2], y)
                    psc = psA.tile([128, 2], F32, tag="big", name="psc")
                    nc.tensor.matmul(psc[:], gt8_sb[:], grp[:],
                                     start=True, stop=True)
                    s_c = small.tile([128, 1], F32, tag="small", name="s_c")
                    nc.vector.tensor_mul(s_c[:], psc[:, 1:2], gnw_sb[:, k:k + 1])
                    t1 = small.tile([128, 1], F32, tag="small", name="t1")
                    nc.vector.tensor_mul(t1[:], psc[:, 0:1], s_c[:])
                    b_c = small.tile([128, 1], F32, tag="small", name="b_c")
                    nc.vector.tensor_sub(b_c[:], gnb_sb[:, k:k + 1], t1[:])
                    ht = singles.tile([128, T], BF16, tag=f"h{k}", name=f"h{k}")
                    for nb in range(NB):
                        sl = slice(512 * nb, 512 * (nb + 1))
                        nc.vector.tensor_scalar(
                            out=ht[:, sl], in0=x_sb[k][:, sl], scalar1=s_c[:],
                            scalar2=b_c[:], op0=MUL, op1=ADD)
                    h_sb.append(ht)

                # ================= qk tiles =================
                qk_tiles = {}

                def gen_qk01():
                    # m = 0, 1 interleaved nb-major so QK(0) steps on the
                    # first t-half can start as early as possible
                    pqs = [psA.tile([128, T], F32, tag="big", name="pq")
                           for _ in range(2)]
                    for m in range(2):
                        qk_tiles[m] = singles.tile(
                            [128, T], BF16, tag=f"qk{m}{sfx}", name=f"qk{m}")
                    for nb in range(NB):
                        sl = slice(512 * nb, 512 * (nb + 1))
                        for m in range(2):
                            for k in range(NCHUNK):
                                nc.tensor.matmul(
                                    pqs[m][:, sl], wqk_ap(k, m),
                                    h_sb[k][:, sl], start=(k == 0),
                                    stop=(k == 3))
                        nc.scalar.activation(qk_tiles[0][:, sl], pqs[0][:, sl],
                                             IDENT, bias=bqk_sb[:, 0:1])
                        nc.vector.tensor_scalar(
                            out=qk_tiles[1][:, sl], in0=pqs[1][:, sl],
                            scalar1=bqk_sb[:, 1:2], scalar2=None, op0=ADD)

                gen_qk01()

                # ================= attention state =================
                ew_pairs = {}

                def _ew(h, j):
                    if (h, j) not in ew_pairs:
                        ew_pairs[(h, j)] = ewp.tile([128, 2, T], FP8, tag="ew",
                                                    name=f"ew{h}_{j}")
                    return ew_pairs[(h, j)]

                def emit_qk_step(h, st):
                    # logits for head h, s-tile st: 2 matmuls + 1 merged exp
                    p, slot = h // 2, h % 2
                    lo, hi = CH * slot, CH * (slot + 1)
                    ktile, qtile = qk_tiles[2 * p], qk_tiles[2 * p + 1]
                    j, pl = st // 2, st % 2
                    pw = psB.tile([128, T], F32, tag="pw", name="pw")
                    for nb in range(NB):
                        nc.tensor.matmul(
                            pw[:, 512 * nb:512 * (nb + 1)],
                            ktile[lo:hi, 128 * st:128 * (st + 1)],
                            qtile[lo:hi, 512 * nb:512 * (nb + 1)],
                            start=True, stop=True)
                    nc.scalar.activation(_ew(h, j)[:, pl, :], pw[:], EXP)

                def emit_qk_half_step(h, st, nb):
                    # one t-half of head h's logits (used to stretch head 7's
                    # exp stream over the last two windows)
                    p, slot = h // 2, h % 2
                    lo, hi = CH * slot, CH * (slot + 1)
                    ktile, qtile = qk_tiles[2 * p], qk_tiles[2 * p + 1]
                    j, pl = st // 2, st % 2
                    sl = slice(512 * nb, 512 * (nb + 1))
                    pw = psB.tile([128, 512], F32, tag="pw", name="pwh")
                    nc.tensor.matmul(
                        pw[:], ktile[lo:hi, 128 * st:128 * (st + 1)],
                        qtile[lo:hi, sl], start=True, stop=True)
                    nc.scalar.activation(_ew(h, j)[:, pl, sl], pw[:], EXP)

                # vt pair tiles (fp8, rider block cols 0:RID with col0 = ones)
                vt_sb = [singles.tile([128, 2, H, RID + CH], FP8,
                                      tag=f"vt{j}", name=f"vt{j}")
                         for j in range(NJ)]
                for j in range(NJ):
                    nc.vector.memset(vt_sb[j][:, :, :, 0:RID], 0.0)
                    nc.vector.memset(vt_sb[j][:, :, :, 0:1], 1.0)

                def emit_v_tile(st):
                    pv = psA.tile([128, C], F32, tag="big", name="pv")
                    for k in range(NCHUNK):
                        nc.tensor.matmul(pv[:],
                                         h_sb[k][:, 128 * st:128 * (st + 1)],
                                         wv_ap(k), start=(k == 0),
                                         stop=(k == 3))
                    nc.vector.tensor_copy(
                        vt_sb[st // 2][:, st % 2, :, RID:RID + CH],
                        pv[:].rearrange("p (h c) -> p h c", h=H))

                a_sb = [singles.tile([128, T], BF16, tag=f"a{p}",
                                     name=f"a{p}{sfx}") for p in range(NCHUNK)]
                acc_sb = [singles.tile([128, T], F32, tag=f"acc{m}",
                                       name=f"acc{m}{sfx}")
                          for m in range(NCHUNK)]

                # ---- prologue: v tiles + QK(0) steps (pv uses the big tag
                # ---- so the pw rotation stays a pure QK/exp double-buffer)
                for st in range(NT):
                    emit_qk_step(0, st)
                    emit_v_tile(st)

                # ================= duties =================
                def qk_spread_duty(m):
                    pq = psA.tile([128, T], F32, tag="big", name="pq")
                    qk = singles.tile([128, T], BF16, tag=f"qk{m}{sfx}",
                                      name=f"qk{m}")
                    qk_tiles[m] = qk
                    for nb in range(NB):
                        sl = slice(512 * nb, 512 * (nb + 1))
                        for k in range(NCHUNK):
                            nc.tensor.matmul(
                                pq[:, sl], wqk_ap(k, m), h_sb[k][:, sl],
                                start=(k == 0), stop=(k == 3))
                            yield
                        nc.vector.tensor_scalar(
                            out=qk[:, sl], in0=pq[:, sl],
                            scalar1=bqk_sb[:, m:m + 1], scalar2=None, op0=ADD)
                    yield

                def proj01_duty():
                    # acc[m] = (Wp0 @ a0 + Wp1 @ a1 + bproj) + x
                    for m in range(NCHUNK):
                        po = psA.tile([128, T], F32, tag="big", name="po")
                        for nb in range(NB):
                            sl = slice(512 * nb, 512 * (nb + 1))
                            nc.tensor.matmul(
                                po[:, sl], wproj_ap(0, m), a_sb[0][:, sl],
                                start=True, stop=False)
                            yield
                            nc.tensor.matmul(
                                po[:, sl], wproj_ap(1, m), a_sb[1][:, sl],
                                start=False, stop=True)
                            yield
                        nc.vector.scalar_tensor_tensor(
                            out=acc_sb[m][:], in0=po[:],
                            scalar=bproj_sb[:, m:m + 1], in1=x_sb[m][:],
                            op0=ADD, op1=ADD)
                        yield

                def proj2_duty(mlo, mhi):
                    # acc[m] += Wp2 @ a2
                    for m in range(mlo, mhi):
                        po = psA.tile([128, T], F32, tag="big", name="po")
                        for nb in range(NB):
                            sl = slice(512 * nb, 512 * (nb + 1))
                            nc.tensor.matmul(
                                po[:, sl], wproj_ap(2, m), a_sb[2][:, sl],
                                start=True, stop=True)
                            yield
                        nc.vector.tensor_add(acc_sb[m][:], po[:], acc_sb[m][:])
                        yield

                # ================= head loop =================
                last_rep = rep == n_reps - 1
                for h in range(H):
                    p, slot = h // 2, h % 2
                    lo, hi = CH * slot, CH * (slot + 1)
                    duties = []
                    if h == 0:
                        duties.append(qk_spread_duty(2))
                        duties.append(qk_spread_duty(3))
                    elif h == 1:
                        duties.append(qk_spread_duty(4))
                    elif h == 2:
                        duties.append(qk_spread_duty(5))
                    elif h == 3:
                        duties.append(qk_spread_duty(6))
                    elif h == 4:
                        duties.append(qk_spread_duty(7))
                    elif h == 5:
                        duties.append(proj01_duty())
                    elif h == 6:
                        duties.append(proj2_duty(0, 2))
                    pa = psA.tile([128, T], F32, tag="big", name="pa")

                    def tail_nb(nb, pa=pa):
                        # normalize head 7's nb half, then pair-3 proj + out
                        sl = slice(512 * nb, 512 * (nb + 1))
                        rcb = recp.tile([1, 512], F32, tag="rcb", name="rcb")
                        nc.vector.reciprocal_approx_fast(rcb[:], pa[0:1, sl])
                        rbb = rbp.tile([CH, 512], F32, tag="rbb", name="rbb")
                        nc.gpsimd.partition_broadcast(rbb[:], rcb[:])
                        nc.vector.tensor_mul(
                            a_sb[3][CH:128, sl], pa[RID:RID + CH, sl], rbb[:])
                        yield
                        for m in range(NCHUNK):
                            po = psA.tile([128, 512], F32, tag="big",
                                          name="pot")
                            nc.tensor.matmul(po[:], wproj_ap(3, m),
                                             a_sb[3][:, sl],
                                             start=True, stop=True)
                            nc.vector.tensor_add(acc_sb[m][:, sl], po[:],
                                                 acc_sb[m][:, sl])
                            if last_rep:
                                nc.sync.dma_start(
                                    out_d[128 * m:128 * (m + 1), sl],
                                    acc_sb[m][:, sl])
                            yield

                    if h == 7:
                        duties.append(proj2_duty(2, 4))
                        # nb0 logits were made during window 6: all nb0 AV now
                        for j in range(NJ):
                            nc.tensor.matmul(
                                pa[:, 0:512], vt_sb[j][:, :, 7, :],
                                ew_pairs[(7, j)][:, :, 0:512],
                                start=(j == 0), stop=(j == NJ - 1),
                                perf_mode=DR)
                        duties.append(tail_nb(0))
                    for st in range(NT):
                        if h < 6:
                            emit_qk_step(h + 1, st)
                        elif h == 6:
                            emit_qk_half_step(7, st, 0)
                        else:
                            emit_qk_half_step(7, st, 1)
                        advanced = 0
                        while duties and advanced < 3:
                            try:
                                next(duties[0])
                                advanced += 1
                            except StopIteration:
                                duties.pop(0)
                        if st % 2 == 1:
                            j = st // 2
                            if h == 7:
                                ewt = ew_pairs.pop((7, j))
                                nc.tensor.matmul(
                                    pa[:, 512:1024], vt_sb[j][:, :, 7, :],
                                    ewt[:, :, 512:1024],
                                    start=(j == 0), stop=(j == NJ - 1),
                                    perf_mode=DR)
                            else:
                                ewt = ew_pairs.pop((h, j))
                                for nb in range(NB):
                                    nc.tensor.matmul(
                                        pa[:, 512 * nb:512 * (nb + 1)],
                                        vt_sb[j][:, :, h, :],
                                        ewt[:, :, 512 * nb:512 * (nb + 1)],
                                        start=(j == 0), stop=(j == NJ - 1),
                                        perf_mode=DR)
                    for g in duties:
                        for _ in g:
                            pass
                    # ---- normalize: a_hat = pa[RID:] / pa[0] ----
                    if h == 7:
                        for _ in tail_nb(1):
                            pass
                    else:
                        rec = recp.tile([1, T], F32, tag="rec", name="rec")
                        nc.vector.reciprocal_approx_fast(rec[:], pa[0:1, :])
                        rb = rbp.tile([CH, T], F32, tag="rb", name="rb")
                        nc.gpsimd.partition_broadcast(rb[:], rec[:])
                        nc.vector.tensor_mul(a_sb[p][lo:hi, :],
                                             pa[RID:RID + CH, :], rb[:])

    nc.compile()
    return nc


def _get_program(n_reps=1):
    key = ("prog", n_reps)
    if key not in _CACHE:
        _CACHE[key] = _build_program(n_reps)
    return _CACHE[key]


def kernel(x, gn_w, gn_b, qkv_w, qkv_b, proj_w, proj_b, _n_reps=1):
    x = np.asarray(x, dtype=np.float32)
    hw = _host_weights(np.asarray(gn_w, np.float32), np.asarray(gn_b, np.float32),
                       np.asarray(qkv_w, np.float32), np.asarray(qkv_b, np.float32),
                       np.asarray(proj_w, np.float32), np.asarray(proj_b, np.float32))
    xr = np.ascontiguousarray(x.reshape(B, C, T))
    nc = _get_program(_n_reps)
    in_maps = [dict(hw, x=xr[b]) for b in range(B)]
    res = run_bass_kernel_spmd(nc, in_maps, core_ids=list(range(B)))
    out = np.stack([res.results[b]["out"] for b in range(B)])
    return out.reshape(B, C, HS, WS).astype(np.float32)
